# revision 28
# baseline (speedup 1.0000x reference)
"""Trainium2 Bass kernel for a GPT-style decoder block (B=2, T=2048, d=1024,
16 heads, FFN 4096), distributed over 8 NeuronCores.

Sharding: DP2 (batch) x TP4 (4 heads + proj-row split per core). The single
collective is a per-token-quarter ReduceScatter of the attention projection
partials over each 4-core group; after it, every core owns its token strips
and runs LN2+FFN (full hidden dim) on just those, writing its 512-token
output slice. The last quarter's ReduceScatter is split in two so only a
256-token collective remains on the critical tail.

v3: fp16 matmul operands throughout (PSUM stays fp32); LN1+QKV pipelined
per 512-token span; attention scores run 3 blocks ahead of the
exp->mask->PV chain so the PE never waits on the softmax; softmax
normalization via partition_broadcast + DVE divide (no PE/ACT involvement);
fc1 weights preloaded to SBUF during attention.

Self-contained: hardcodes all shapes; no sibling imports.
"""
import numpy as np

import concourse.bacc as bacc
import concourse.mybir as mybir
import concourse.tile as tile
from concourse.bass_utils import run_bass_kernel_spmd
from concourse.masks import make_identity

F32 = mybir.dt.float32
F16 = mybir.dt.float16
AF = mybir.ActivationFunctionType
OP = mybir.AluOpType

P = 128
T = 2048          # tokens per batch element
D = 1024          # embed dim
NT = T // P       # 16 token tiles
DC = D // P       # 8 d-chunks
FH = 4            # heads per core
DH = 64           # head dim
FQ = 256          # q (=k=v) features per core
HID = 4096        # full FFN hidden
TS = 512          # token slice per core
NQ = 4            # token quarters
SPAN = 512        # attention query span
NSPAN = T // SPAN
EPS = 1e-5
GROUPS = [[0, 1, 2, 3], [4, 5, 6, 7]]
WFC_PRE = 16      # fc1 hid-tiles preloaded to SBUF (rest streamed)
SKEW = 3          # scores run this many blocks ahead of PV


def build_nc():
    nc = bacc.Bacc(None, target_bir_lowering=False)

    # ---- external I/O ----
    x_d = nc.dram_tensor("x", [T, D], F16, kind="ExternalInput")
    xs_d = nc.dram_tensor("xs", [TS, D], F16, kind="ExternalInput")
    wq_d = nc.dram_tensor("wq", [D, FQ], F16, kind="ExternalInput")
    wk_d = nc.dram_tensor("wk", [D, FQ], F16, kind="ExternalInput")
    wv_d = nc.dram_tensor("wv", [D, FQ], F16, kind="ExternalInput")
    bqk_d = nc.dram_tensor("bqk", [P, 4], F32, kind="ExternalInput")
    bvb_d = nc.dram_tensor("bvb", [P, FQ], F32, kind="ExternalInput")
    wp_d = nc.dram_tensor("wp", [FQ, D], F16, kind="ExternalInput")
    bpb_d = nc.dram_tensor("bpb", [P, D], F32, kind="ExternalInput")
    g1_d = nc.dram_tensor("g1", [P, DC], F32, kind="ExternalInput")
    b1_d = nc.dram_tensor("b1", [P, DC], F32, kind="ExternalInput")
    g2_d = nc.dram_tensor("g2", [P, DC], F32, kind="ExternalInput")
    b2_d = nc.dram_tensor("b2", [P, DC], F32, kind="ExternalInput")
    wfc_d = nc.dram_tensor("wfc", [HID // P, P, DC, P], F16,
                           kind="ExternalInput")
    bfc_d = nc.dram_tensor("bfc", [P, HID // P], F32, kind="ExternalInput")
    wfc2_d = nc.dram_tensor("wfc2", [HID, D], F16, kind="ExternalInput")
    bfc2b_d = nc.dram_tensor("bfc2b", [P, D], F32, kind="ExternalInput")
    out_d = nc.dram_tensor("out", [TS, D], F32, kind="ExternalOutput")

    rs_in = [nc.dram_tensor(f"rs_in{q}", [SPAN, D], F16) for q in range(NQ)]
    rs_out = [nc.dram_tensor(f"rs_out{q}", [P, D], F16) for q in range(3)]
    rs_out3 = [nc.dram_tensor(f"rs_out3{i}", [P // 2, D], F16)
               for i in range(2)]

    with tile.TileContext(nc) as tc:
        cst_cm = tc.tile_pool(name="cst", bufs=1)
        cst = cst_cm.__enter__()
        big_cm = tc.tile_pool(name="big", bufs=1)
        big = big_cm.__enter__()

        # ---- constants / small params ----
        identf = cst.tile([P, P], F32)
        make_identity(nc, identf[:])
        ident = cst.tile([P, P], F16)
        nc.vector.tensor_copy(ident[:], identf[:])
        ones4 = cst.tile([P, FH, 1], F16)
        with nc.allow_low_precision("exact value 1.0"):
            nc.gpsimd.memset(ones4[:], 1.0)
        epsb = cst.tile([P, 1], F32)
        nc.gpsimd.memset(epsb[:], EPS)

        bqkt = cst.tile([P, 4], F32)
        nc.sync.dma_start(bqkt[:], bqk_d[:])
        g1t = cst.tile([P, DC], F32)
        nc.sync.dma_start(g1t[:], g1_d[:])
        b1t = cst.tile([P, DC], F32)
        nc.sync.dma_start(b1t[:], b1_d[:])
        g2t = cst.tile([P, DC], F32)
        nc.sync.dma_start(g2t[:], g2_d[:])
        b2t = cst.tile([P, DC], F32)
        nc.sync.dma_start(b2t[:], b2_d[:])
        bfct = cst.tile([P, HID // P], F32)
        nc.sync.dma_start(bfct[:], bfc_d[:])
        bvb = cst.tile([P, FQ], F32)
        nc.sync.dma_start(bvb[:], bvb_d[:])
        bpb = cst.tile([P, D], F32)
        nc.sync.dma_start(bpb[:], bpb_d[:])
        bfc2b = cst.tile([P, D], F32)
        nc.sync.dma_start(bfc2b[:], bfc2b_d[:])

        # ---- resident weights ----
        watt_cm = tc.tile_pool(name="watt", bufs=1)
        watt = watt_cm.__enter__()
        wq_t = watt.tile([P, DC, FQ], F16)
        wk_t = watt.tile([P, DC, FQ], F16)
        wv_t = watt.tile([P, DC, FQ], F16)
        wp_t = watt.tile([P, 2, D], F16)
        nc.sync.dma_start(wq_t[:], wq_d[:, :].rearrange("(c p) f -> p c f", p=P))
        nc.sync.dma_start(wk_t[:], wk_d[:, :].rearrange("(c p) f -> p c f", p=P))
        nc.sync.dma_start(wv_t[:], wv_d[:, :].rearrange("(c p) f -> p c f", p=P))
        nc.sync.dma_start(wp_t[:], wp_d[:, :].rearrange("(c p) f -> p c f", p=P))
        # fc1 weight, mostly preloaded (DMA issued at attention start);
        # lives in `big` so it survives until the FFN phase
        wfc_sb = big.tile([P, WFC_PRE, DC, P], F16, name="wfc_sb")

        # ---- big tag-shared tiles (phase 1-4 lives) ----
        h1T = big.tile([P, DC, T], F16, tag="A", name="h1T")
        qT = big.tile([P, 2, T], F16, tag="B1", name="qT")
        kT = big.tile([P, 2, T], F16, tag="B2", name="kT")
        vhat = big.tile([P, NT, FH * (DH + 1)], F16, tag="B3", name="vhat")
        attT = big.tile([P, 2, T], F16, tag="B4", name="attT")

        # ============ Front: LN1 + transpose + QKV, pipelined per span ========
        qk_meta = [(wq_t, 0, qT, 0), (wq_t, 1, qT, 1),
                   (wk_t, 0, kT, 0), (wk_t, 1, kT, 1)]
        with tc.tile_pool(name="p_ln", bufs=1) as p_ln, \
             tc.tile_pool(name="p_xq", bufs=2) as p_xq, \
             tc.tile_pool(name="p_xh", bufs=3) as p_xh, \
             tc.tile_pool(name="ps_t1", bufs=2, space="PSUM") as ps_t1, \
             tc.tile_pool(name="ps_qk", bufs=3, space="PSUM") as ps_qk, \
             tc.tile_pool(name="ps_v", bufs=2, space="PSUM") as ps_v:
            bn6 = p_ln.tile([P, NT, 2, 6], F32)
            mv = p_ln.tile([P, NT, 2], F32)
            sd = p_ln.tile([P, NT], F32)
            rstd = p_ln.tile([P, NT], F32)
            m2 = p_ln.tile([P, NT], F32)
            xsrc = x_d[:, :].rearrange("(t p) d -> p t d", p=P)
            for s in range(NSPAN):
                for tgl in range(2):
                    tg = 2 * s + tgl
                    xq = p_xq.tile([P, 2, D], F16, tag="xq", name=f"xq{tg}")
                    if tg == 0:
                        for tt in range(2):
                            nc.sync.dma_start(xq[:, tt:tt + 1, :],
                                              xsrc[:, tt:tt + 1, :])
                    else:
                        nc.sync.dma_start(xq[:],
                                          xsrc[:, tg * 2:(tg + 1) * 2, :])
                    xhs = []
                    for tt in range(2):
                        t = tg * 2 + tt
                        for a in range(2):
                            nc.vector.bn_stats(
                                bn6[:, t, a, :],
                                xq[:, tt, a * 512:(a + 1) * 512])
                        nc.vector.bn_aggr(mv[:, t, :], bn6[:, t, :, :])
                        nc.scalar.activation(sd[:, t:t + 1], mv[:, t, 1:2],
                                             AF.Sqrt, bias=epsb[:])
                        nc.vector.reciprocal(rstd[:, t:t + 1], sd[:, t:t + 1])
                        nc.vector.tensor_scalar(
                            m2[:, t:t + 1], mv[:, t, 0:1], rstd[:, t:t + 1],
                            -1.0, OP.mult, OP.mult)
                        xh = p_xh.tile([P, D], F16, tag="xh", name=f"xh{t}")
                        with nc.allow_low_precision("fp16 activations"):
                            nc.vector.tensor_scalar(
                                xh[:], xq[:, tt, :], mv[:, t, 0:1],
                                rstd[:, t:t + 1], OP.subtract, OP.mult)
                        xhs.append(xh)
                    for j in range(DC):
                        pt = ps_t1.tile([P, 2 * P], F16, tag="t1",
                                        name=f"t1_{tg}_{j}")
                        for tt in range(2):
                            nc.tensor.transpose(
                                pt[:, tt * P:(tt + 1) * P],
                                xhs[tt][:, j * P:(j + 1) * P], ident[:])
                        dst = h1T[:, j, tg * 2 * P:(tg + 1) * 2 * P]
                        with nc.allow_low_precision("fp16 activations"):
                            if j % 2 == 0:
                                nc.scalar.activation(
                                    dst, pt[:], AF.Identity,
                                    bias=b1t[:, j:j + 1], scale=g1t[:, j:j + 1])
                            else:
                                nc.vector.tensor_scalar(
                                    dst, pt[:], g1t[:, j:j + 1],
                                    b1t[:, j:j + 1], OP.mult, OP.add)
                # QKV for this span
                for fb in range(4):
                    wsrc, half, dest, dhalf = qk_meta[fb]
                    pq = ps_qk.tile([P, SPAN], F32, tag="qk",
                                    name=f"qk{fb}_{s}")
                    for kc in range(DC):
                        nc.tensor.matmul(
                            pq[:], wsrc[:, kc, half * P:(half + 1) * P],
                            h1T[:, kc, s * SPAN:(s + 1) * SPAN],
                            start=(kc == 0), stop=(kc == DC - 1))
                    with nc.allow_low_precision("fp16 activations"):
                        nc.scalar.activation(
                            dest[:, dhalf, s * SPAN:(s + 1) * SPAN], pq[:],
                            AF.Identity, bias=bqkt[:, fb:fb + 1])
                for m in range(4 * s, 4 * s + 4):
                    pv = ps_v.tile([P, FQ], F32, tag="v", name=f"v{m}")
                    for kc in range(DC):
                        nc.tensor.matmul(
                            pv[:], h1T[:, kc, m * P:(m + 1) * P],
                            wv_t[:, kc, :],
                            start=(kc == 0), stop=(kc == DC - 1))
                    vdst = vhat[:, m, :].rearrange("p (h x) -> p h x",
                                                   x=DH + 1)
                    with nc.allow_low_precision("fp16 activations"):
                        nc.vector.tensor_tensor(
                            vdst[:, :, 0:DH],
                            pv[:].rearrange("p (h x) -> p h x", x=DH),
                            bvb[:].rearrange("p (h x) -> p h x", x=DH),
                            OP.add)
                        nc.vector.tensor_copy(vdst[:, :, DH:DH + 1], ones4[:])

        # prefetch fc1 weights into SBUF while attention computes; one DMA
        # per 256KB tile so no single queue builds a deep backlog ahead of
        # the ReduceScatter input writes
        for pm in range(WFC_PRE):
            nc.sync.dma_start(
                wfc_sb[:, pm:pm + 1, :, :],
                wfc_d[pm:pm + 1].rearrange("m p c f -> p m c f"))

        # ================= Attention + proj + ReduceScatter =================
        # span-outer; scores run SKEW blocks ahead of the exp->mask->PV chain
        with tc.tile_pool(name="p_e", bufs=1) as p_e, \
             tc.tile_pool(name="p_pr", bufs=3) as p_pr, \
             tc.tile_pool(name="ps_s", bufs=4, space="PSUM") as ps_s, \
             tc.tile_pool(name="ps_pv", bufs=2, space="PSUM") as ps_pv, \
             tc.tile_pool(name="ps_pr", bufs=2, space="PSUM") as ps_pr:
            for s in range(NSPAN):
                nkb = (s + 1) * (SPAN // P)
                for h in range(FH):
                    hp, ro = h // 2, (h % 2) * DH
                    ppv = ps_pv.tile([DH + 1, SPAN], F32, tag="pv",
                                     name=f"pv{h}_{s}")
                    es = {}

                    def scores(kb, h=h, s=s, hp=hp, ro=ro, es=es):
                        pst = ps_s.tile([P, SPAN], F32, tag="sT",
                                        name=f"sT{h}_{s}_{kb}")
                        nc.tensor.matmul(
                            pst[:],
                            kT[ro:ro + DH, hp, kb * P:(kb + 1) * P],
                            qT[ro:ro + DH, hp, s * SPAN:(s + 1) * SPAN])
                        e = p_e.tile([P, SPAN], F16, tag="e", bufs=SKEW + 2,
                                     name=f"e{h}_{s}_{kb}")
                        with nc.allow_low_precision("fp16 activations"):
                            nc.scalar.activation(e[:], pst[:], AF.Exp,
                                                 scale=0.125)
                        j = kb - s * (SPAN // P)
                        if j >= 0:
                            # zero the causally-masked region in place
                            nc.gpsimd.affine_select(
                                out=e[:], in_=e[:], compare_op=OP.is_ge,
                                fill=0.0, base=-j * P, channel_multiplier=-1,
                                pattern=[[1, SPAN]])
                        es[kb] = e

                    def pv_acc(kb, h=h, nkb=nkb, ppv=ppv, es=es):
                        nc.tensor.matmul(
                            ppv[:],
                            vhat[:, kb, h * (DH + 1):(h + 1) * (DH + 1)],
                            es.pop(kb)[:], start=(kb == 0),
                            stop=(kb == nkb - 1))

                    for i in range(nkb + SKEW):
                        if i < nkb:
                            scores(i)
                        if i >= SKEW:
                            pv_acc(i - SKEW)
                    # normalize: att = pv / den  (den broadcast down 64 rows)
                    den = p_e.tile([1, SPAN], F32, tag="den", bufs=2,
                                   name=f"den{h}_{s}")
                    nc.vector.tensor_copy(den[:], ppv[DH:DH + 1, :])
                    dbs = p_e.tile([DH, SPAN], F32, tag="dbs", bufs=2,
                                   name=f"dbs{h}_{s}")
                    nc.gpsimd.partition_broadcast(dbs[:], den[:], channels=DH)
                    rbs = p_e.tile([DH, SPAN], F32, tag="rbs", bufs=2,
                                   name=f"rbs{h}_{s}")
                    nc.vector.reciprocal(rbs[:], dbs[:])
                    with nc.allow_low_precision("fp16 activations"):
                        nc.vector.tensor_tensor(
                            attT[ro:ro + DH, hp, s * SPAN:(s + 1) * SPAN],
                            ppv[0:DH, :], rbs[:], OP.mult)

                # projection partial for this token quarter, then its RS
                for mtl in range(4):
                    m = s * 4 + mtl
                    for n in range(2):
                        pp = ps_pr.tile([P, SPAN], F32, tag="proj",
                                        name=f"pr{m}_{n}")
                        for kc in range(2):
                            nc.tensor.matmul(
                                pp[:], attT[:, kc, m * P:(m + 1) * P],
                                wp_t[:, kc, n * SPAN:(n + 1) * SPAN],
                                start=(kc == 0), stop=(kc == 1))
                        pe = p_pr.tile([P, SPAN], F16, tag="pe",
                                       name=f"pe{m}_{n}")
                        with nc.allow_low_precision("fp16 rs payload"):
                            nc.vector.tensor_copy(pe[:], pp[:])
                        nc.sync.dma_start(
                            rs_in[s][mtl * P:(mtl + 1) * P,
                                     n * SPAN:(n + 1) * SPAN], pe[:])
                    if s == 3 and mtl == 1:
                        nc.gpsimd.collective_compute(
                            "ReduceScatter", OP.add, replica_groups=GROUPS,
                            ins=[rs_in[3][0:SPAN // 2, :]],
                            outs=[rs_out3[0][:]])
                if s < 3:
                    nc.gpsimd.collective_compute(
                        "ReduceScatter", OP.add, replica_groups=GROUPS,
                        ins=[rs_in[s][:]], outs=[rs_out[s][:]])
                else:
                    nc.gpsimd.collective_compute(
                        "ReduceScatter", OP.add, replica_groups=GROUPS,
                        ins=[rs_in[3][SPAN // 2:, :]], outs=[rs_out3[1][:]])
        watt_cm.__exit__(None, None, None)

        # ================= Phase 5: residual + LN2 =================
        x2 = big.tile([P, NQ, D], F32, tag="B1", name="x2")
        h2T = big.tile([P, DC, TS], F16, tag="B2", name="h2T")
        xs_t = big.tile([P, NQ, D], F16, tag="B3", name="xs_t")
        xh2 = big.tile([P, NQ, D], F16, tag="B4", name="xh2")
        nc.sync.dma_start(xs_t[:], xs_d[:, :].rearrange("(q p) d -> p q d", p=P))

        with tc.tile_pool(name="p_l2", bufs=1) as p_l2, \
             tc.tile_pool(name="p_rs", bufs=2) as p_rs, \
             tc.tile_pool(name="ps_t2", bufs=2, space="PSUM") as ps_t2:
            bn6b = p_l2.tile([P, NQ, 2, 6], F32)
            mvb = p_l2.tile([P, NQ, 2], F32)
            sdb = p_l2.tile([P, NQ], F32)
            rstdb = p_l2.tile([P, NQ], F32)
            m2b = p_l2.tile([P, NQ], F32)
            for q in range(NQ):
                rst = p_rs.tile([P, D], F16, tag="rst", name=f"rst{q}")
                if q < 3:
                    nc.sync.dma_start(rst[:], rs_out[q][:])
                else:
                    nc.sync.dma_start(rst[0:P // 2, :], rs_out3[0][:])
                    nc.sync.dma_start(rst[P // 2:, :], rs_out3[1][:])
                nc.vector.tensor_tensor(x2[:, q, :], rst[:],
                                        xs_t[:, q, :], OP.add)
                nc.vector.tensor_tensor(x2[:, q, :], x2[:, q, :], bpb[:],
                                        OP.add)
                for a in range(2):
                    nc.vector.bn_stats(
                        bn6b[:, q, a, :], x2[:, q, a * 512:(a + 1) * 512])
                nc.vector.bn_aggr(mvb[:, q, :], bn6b[:, q, :, :])
                nc.scalar.activation(sdb[:, q:q + 1], mvb[:, q, 1:2], AF.Sqrt,
                                     bias=epsb[:])
                nc.vector.reciprocal(rstdb[:, q:q + 1], sdb[:, q:q + 1])
                nc.vector.tensor_scalar(
                    m2b[:, q:q + 1], mvb[:, q, 0:1], rstdb[:, q:q + 1], -1.0,
                    OP.mult, OP.mult)
                with nc.allow_low_precision("fp16 activations"):
                    nc.scalar.activation(
                        xh2[:, q, :], x2[:, q, :], AF.Identity,
                        bias=m2b[:, q:q + 1], scale=rstdb[:, q:q + 1])
            for hf in range(2):
                for j in range(DC):
                    pt = ps_t2.tile([P, TS // 2], F16, tag="t2",
                                    name=f"t2_{hf}_{j}")
                    for qq in range(2):
                        q = hf * 2 + qq
                        nc.tensor.transpose(
                            pt[:, qq * P:(qq + 1) * P],
                            xh2[:, q, j * P:(j + 1) * P], ident[:])
                    with nc.allow_low_precision("fp16 activations"):
                        nc.scalar.activation(
                            h2T[:, j, hf * 256:(hf + 1) * 256], pt[:],
                            AF.Identity, bias=b2t[:, j:j + 1],
                            scale=g2t[:, j:j + 1])

        # ================= Phase 6: FFN =================
        aT = big.tile([P, HID // P, TS], F16, tag="A", name="aT")
        with tc.tile_pool(name="p_wf", bufs=4) as p_wf, \
             tc.tile_pool(name="ps_f1", bufs=2, space="PSUM") as ps_f1:
            for m in range(HID // P):
                if m < WFC_PRE:
                    stat = lambda kc, m=m: wfc_sb[:, m, kc, :]
                else:
                    wmt = p_wf.tile([P, DC, P], F16, tag="wfc",
                                    name=f"wfc{m}")
                    nc.sync.dma_start(wmt[:], wfc_d[m])
                    stat = lambda kc, w=wmt: w[:, kc, :]
                pf = ps_f1.tile([P, TS], F32, tag="f1", name=f"f1_{m}")
                for kc in range(DC):
                    nc.tensor.matmul(pf[:], stat(kc), h2T[:, kc, :],
                                     start=(kc == 0), stop=(kc == DC - 1))
                with nc.allow_low_precision("fp16 activations"):
                    nc.scalar.activation(aT[:, m, :], pf[:], AF.Gelu,
                                         bias=bfct[:, m:m + 1])

        with tc.tile_pool(name="p_w2", bufs=6) as p_w2, \
             tc.tile_pool(name="p_ot", bufs=3) as p_ot, \
             tc.tile_pool(name="ps_f2", bufs=4, space="PSUM") as ps_f2:
            p4s = [ps_f2.tile([P, 2, SPAN], F32, tag="f2", name=f"f2_{mt}")
                   for mt in range(NQ)]
            for kc in range(HID // P):
                w2 = p_w2.tile([P, D], F16, tag="w2", name=f"w2_{kc}")
                nc.sync.dma_start(w2[:], wfc2_d[kc * P:(kc + 1) * P, :])
                for mt in range(NQ):
                    for n in range(2):
                        nc.tensor.matmul(
                            p4s[mt][:, n, :],
                            aT[:, kc, mt * P:(mt + 1) * P],
                            w2[:, n * SPAN:(n + 1) * SPAN],
                            start=(kc == 0), stop=(kc == HID // P - 1))
            for mt in range(NQ):
                for n in range(2):
                    ot = p_ot.tile([P, SPAN], F32, tag="ot",
                                   name=f"ot{n}_{mt}")
                    nc.vector.tensor_tensor(
                        ot[:], p4s[mt][:, n, :],
                        x2[:, mt, n * SPAN:(n + 1) * SPAN], OP.add)
                    nc.vector.tensor_tensor(
                        ot[:], ot[:],
                        bfc2b[:, n * SPAN:(n + 1) * SPAN], OP.add)
                    nc.sync.dma_start(
                        out_d[mt * P:(mt + 1) * P, n * SPAN:(n + 1) * SPAN],
                        ot[:])

        big_cm.__exit__(None, None, None)
        cst_cm.__exit__(None, None, None)

    nc.finalize()
    return nc


def shard_inputs(inputs):
    """Full inputs -> per-core in_maps (8 cores)."""
    f = lambda a: np.ascontiguousarray(np.asarray(a, dtype=np.float32))
    h = lambda a: np.ascontiguousarray(np.asarray(a, dtype=np.float16))
    x = f(inputs["x"])
    w_attn, b_attn = f(inputs["w_attn"]), f(inputs["b_attn"])
    w_proj, b_proj = f(inputs["w_proj"]), f(inputs["b_proj"])
    ln1_g, ln1_b = f(inputs["ln1_g"]), f(inputs["ln1_b"])
    ln2_g, ln2_b = f(inputs["ln2_g"]), f(inputs["ln2_b"])
    w_fc, b_fc = f(inputs["w_fc"]), f(inputs["b_fc"])
    w_fc2, b_fc2 = f(inputs["w_fc2"]), f(inputs["b_fc2"])

    g1t = np.ascontiguousarray(ln1_g.reshape(DC, P).T)
    b1t = np.ascontiguousarray(ln1_b.reshape(DC, P).T)
    g2t = np.ascontiguousarray(ln2_g.reshape(DC, P).T)
    b2t = np.ascontiguousarray(ln2_b.reshape(DC, P).T)
    wfc_r = h(w_fc.reshape(DC, P, HID // P, P).transpose(2, 1, 0, 3))
    bfct = np.ascontiguousarray(b_fc.reshape(HID // P, P).T)
    bpb = np.ascontiguousarray(np.broadcast_to(b_proj.reshape(1, D), (P, D)))
    bfc2b = np.ascontiguousarray(np.broadcast_to(b_fc2.reshape(1, D), (P, D)))
    wfc2_h = h(w_fc2)

    in_maps = []
    for c in range(8):
        g, tp = c // 4, c % 4
        sl = slice(tp * FQ, (tp + 1) * FQ)
        bq = b_attn[0 * D:1 * D][sl]
        bk = b_attn[1 * D:2 * D][sl]
        bv = b_attn[2 * D:3 * D][sl]
        bqk = np.concatenate([bq.reshape(2, P).T, bk.reshape(2, P).T], axis=1)
        bvb = np.broadcast_to(bv.reshape(1, FQ), (P, FQ))
        # token strips owned by this core: quarters 0-2 are 128-row strips,
        # quarter 3 is two 64-row strips (split ReduceScatter)
        strips = [x[g, q * SPAN + tp * P: q * SPAN + (tp + 1) * P]
                  for q in range(3)]
        strips.append(x[g, 3 * SPAN + tp * (P // 2):
                        3 * SPAN + (tp + 1) * (P // 2)])
        strips.append(x[g, 3 * SPAN + SPAN // 2 + tp * (P // 2):
                        3 * SPAN + SPAN // 2 + (tp + 1) * (P // 2)])
        xs = np.concatenate(strips, axis=0)
        in_maps.append({
            "x": h(x[g]),
            "xs": h(xs),
            "wq": h(w_attn[:, 0 * D:1 * D][:, sl]),
            "wk": h(w_attn[:, 1 * D:2 * D][:, sl]),
            "wv": h(w_attn[:, 2 * D:3 * D][:, sl]),
            "bqk": np.ascontiguousarray(bqk),
            "bvb": np.ascontiguousarray(bvb),
            "wp": h(w_proj[sl, :]),
            "bpb": bpb,
            "g1": g1t, "b1": b1t, "g2": g2t, "b2": b2t,
            "wfc": wfc_r, "bfc": bfct,
            "wfc2": wfc2_h, "bfc2b": bfc2b,
        })
    return in_maps


def assemble(results):
    out = np.empty((2, T, D), dtype=np.float32)
    for c in range(8):
        g, tp = c // 4, c % 4
        r = np.asarray(results[c]["out"])
        for q in range(3):
            out[g, q * SPAN + tp * P: q * SPAN + (tp + 1) * P] = \
                r[q * P:(q + 1) * P]
        out[g, 3 * SPAN + tp * (P // 2):
            3 * SPAN + (tp + 1) * (P // 2)] = r[3 * P:3 * P + P // 2]
        out[g, 3 * SPAN + SPAN // 2 + tp * (P // 2):
            3 * SPAN + SPAN // 2 + (tp + 1) * (P // 2)] = r[3 * P + P // 2:]
    return out


_NC = None


def kernel(**inputs):
    global _NC
    if _NC is None:
        _NC = build_nc()
    in_maps = shard_inputs(inputs)
    res = run_bass_kernel_spmd(_NC, in_maps, list(range(8)))
    return assemble(res.results)


# revision 30
# speedup vs baseline: 1.0660x; 1.0660x over previous
"""Trainium2 Bass kernel for a GPT-style decoder block (B=2, T=2048, d=1024,
16 heads, FFN 4096), distributed over 8 NeuronCores.

Sharding: DP2 (batch) x TP4 (4 heads + proj-row split per core). The single
collective is a per-token-quarter ReduceScatter of the attention projection
partials over each 4-core group; after it, every core owns its token strips
and runs LN2+FFN (full hidden dim) on just those, writing its 512-token
output slice. The last quarter's ReduceScatter is split in two so only a
256-token collective remains on the critical tail.

v3: fp16 matmul operands throughout (PSUM stays fp32); LN1+QKV pipelined
per 512-token span; attention scores run 3 blocks ahead of the
exp->mask->PV chain so the PE never waits on the softmax; softmax
normalization via partition_broadcast + DVE divide (no PE/ACT involvement);
fc1 weights preloaded to SBUF during attention.

Self-contained: hardcodes all shapes; no sibling imports.
"""
import numpy as np

import concourse.bacc as bacc
import concourse.mybir as mybir
import concourse.tile as tile
from concourse.bass_utils import run_bass_kernel_spmd
from concourse.masks import make_identity

F32 = mybir.dt.float32
F16 = mybir.dt.float16
AF = mybir.ActivationFunctionType
OP = mybir.AluOpType

P = 128
T = 2048          # tokens per batch element
D = 1024          # embed dim
NT = T // P       # 16 token tiles
DC = D // P       # 8 d-chunks
FH = 4            # heads per core
DH = 64           # head dim
FQ = 256          # q (=k=v) features per core
HID = 4096        # full FFN hidden
TS = 512          # token slice per core
NQ = 4            # token quarters
SPAN = 512        # attention query span
NSPAN = T // SPAN
EPS = 1e-5
GROUPS = [[0, 1, 2, 3], [4, 5, 6, 7]]
WFC_PRE = 24      # fc1 hid-tiles preloaded to SBUF (rest streamed)
SKEW = 3          # scores run this many blocks ahead of PV


def build_nc():
    nc = bacc.Bacc(None, target_bir_lowering=False)

    # ---- external I/O ----
    x_d = nc.dram_tensor("x", [T, D], F16, kind="ExternalInput")
    xs_d = nc.dram_tensor("xs", [TS, D], F16, kind="ExternalInput")
    wq_d = nc.dram_tensor("wq", [D, FQ], F16, kind="ExternalInput")
    wk_d = nc.dram_tensor("wk", [D, FQ], F16, kind="ExternalInput")
    wv_d = nc.dram_tensor("wv", [D, FQ], F16, kind="ExternalInput")
    bqk_d = nc.dram_tensor("bqk", [P, 4], F32, kind="ExternalInput")
    bvb_d = nc.dram_tensor("bvb", [P, FQ], F32, kind="ExternalInput")
    wp_d = nc.dram_tensor("wp", [FQ, D], F16, kind="ExternalInput")
    bpb_d = nc.dram_tensor("bpb", [P, D], F32, kind="ExternalInput")
    g1_d = nc.dram_tensor("g1", [P, DC], F32, kind="ExternalInput")
    b1_d = nc.dram_tensor("b1", [P, DC], F32, kind="ExternalInput")
    g2_d = nc.dram_tensor("g2", [P, DC], F32, kind="ExternalInput")
    b2_d = nc.dram_tensor("b2", [P, DC], F32, kind="ExternalInput")
    wfc_d = nc.dram_tensor("wfc", [HID // P, P, DC, P], F16,
                           kind="ExternalInput")
    bfc_d = nc.dram_tensor("bfc", [P, HID // P], F32, kind="ExternalInput")
    wfc2_d = nc.dram_tensor("wfc2", [HID, D], F16, kind="ExternalInput")
    bfc2b_d = nc.dram_tensor("bfc2b", [P, D], F32, kind="ExternalInput")
    out_d = nc.dram_tensor("out", [TS, D], F32, kind="ExternalOutput")

    rs_in = [nc.dram_tensor(f"rs_in{q}", [SPAN, D], F16) for q in range(NQ)]
    rs_out = [nc.dram_tensor(f"rs_out{q}", [P, D], F16) for q in range(NQ)]

    with tile.TileContext(nc) as tc:
        cst_cm = tc.tile_pool(name="cst", bufs=1)
        cst = cst_cm.__enter__()
        big_cm = tc.tile_pool(name="big", bufs=1)
        big = big_cm.__enter__()

        # ---- constants / small params ----
        identf = cst.tile([P, P], F32)
        make_identity(nc, identf[:])
        ident = cst.tile([P, P], F16)
        nc.vector.tensor_copy(ident[:], identf[:])
        ones4 = cst.tile([P, FH, 1], F16)
        with nc.allow_low_precision("exact value 1.0"):
            nc.gpsimd.memset(ones4[:], 1.0)
        epsb = cst.tile([P, 1], F32)
        nc.gpsimd.memset(epsb[:], EPS)

        bqkt = cst.tile([P, 4], F32)
        nc.sync.dma_start(bqkt[:], bqk_d[:])
        g1t = cst.tile([P, DC], F32)
        nc.sync.dma_start(g1t[:], g1_d[:])
        b1t = cst.tile([P, DC], F32)
        nc.sync.dma_start(b1t[:], b1_d[:])
        g2t = cst.tile([P, DC], F32)
        nc.sync.dma_start(g2t[:], g2_d[:])
        b2t = cst.tile([P, DC], F32)
        nc.sync.dma_start(b2t[:], b2_d[:])
        bfct = cst.tile([P, HID // P], F32)
        nc.sync.dma_start(bfct[:], bfc_d[:])
        bvb = cst.tile([P, FQ], F32)
        nc.sync.dma_start(bvb[:], bvb_d[:])
        bpb = cst.tile([P, D], F32)
        nc.sync.dma_start(bpb[:], bpb_d[:])
        bfc2b = cst.tile([P, D], F32)
        nc.sync.dma_start(bfc2b[:], bfc2b_d[:])

        # ---- resident weights ----
        watt_cm = tc.tile_pool(name="watt", bufs=1)
        watt = watt_cm.__enter__()
        wq_t = watt.tile([P, DC, FQ], F16)
        wk_t = watt.tile([P, DC, FQ], F16)
        wv_t = watt.tile([P, DC, FQ], F16)
        wp_t = watt.tile([P, 2, D], F16)
        nc.sync.dma_start(wq_t[:], wq_d[:, :].rearrange("(c p) f -> p c f", p=P))
        nc.sync.dma_start(wk_t[:], wk_d[:, :].rearrange("(c p) f -> p c f", p=P))
        nc.sync.dma_start(wv_t[:], wv_d[:, :].rearrange("(c p) f -> p c f", p=P))
        nc.sync.dma_start(wp_t[:], wp_d[:, :].rearrange("(c p) f -> p c f", p=P))
        # fc1 weight, mostly preloaded (DMA issued at attention start);
        # lives in `big` so it survives until the FFN phase
        wfc_sb = big.tile([P, WFC_PRE, DC, P], F16, name="wfc_sb")

        # ---- big tag-shared tiles (phase 1-4 lives) ----
        h1T = big.tile([P, DC, T], F16, tag="A", name="h1T")
        qT = big.tile([P, 2, T], F16, tag="B1", name="qT")
        kT = big.tile([P, 2, T], F16, tag="B2", name="kT")
        vhat = big.tile([P, NT, FH * (DH + 1)], F16, tag="B3", name="vhat")
        attT = big.tile([P, 2, T], F16, tag="B4", name="attT")

        # ============ Front: LN1 + transpose + QKV, pipelined per span ========
        qk_meta = [(wq_t, 0, qT, 0), (wq_t, 1, qT, 1),
                   (wk_t, 0, kT, 0), (wk_t, 1, kT, 1)]
        with tc.tile_pool(name="p_ln", bufs=1) as p_ln, \
             tc.tile_pool(name="p_xq", bufs=2) as p_xq, \
             tc.tile_pool(name="p_xh", bufs=3) as p_xh, \
             tc.tile_pool(name="ps_t1", bufs=2, space="PSUM") as ps_t1, \
             tc.tile_pool(name="ps_qk", bufs=3, space="PSUM") as ps_qk, \
             tc.tile_pool(name="ps_v", bufs=2, space="PSUM") as ps_v:
            bn6 = p_ln.tile([P, NT, 2, 6], F32)
            mv = p_ln.tile([P, NT, 2], F32)
            sd = p_ln.tile([P, NT], F32)
            rstd = p_ln.tile([P, NT], F32)
            m2 = p_ln.tile([P, NT], F32)
            xsrc = x_d[:, :].rearrange("(t p) d -> p t d", p=P)
            for s in range(NSPAN):
                for tgl in range(2):
                    tg = 2 * s + tgl
                    xq = p_xq.tile([P, 2, D], F16, tag="xq", name=f"xq{tg}")
                    if tg == 0:
                        for tt in range(2):
                            nc.sync.dma_start(xq[:, tt:tt + 1, :],
                                              xsrc[:, tt:tt + 1, :])
                    else:
                        nc.sync.dma_start(xq[:],
                                          xsrc[:, tg * 2:(tg + 1) * 2, :])
                    xhs = []
                    for tt in range(2):
                        t = tg * 2 + tt
                        for a in range(2):
                            nc.vector.bn_stats(
                                bn6[:, t, a, :],
                                xq[:, tt, a * 512:(a + 1) * 512])
                        nc.vector.bn_aggr(mv[:, t, :], bn6[:, t, :, :])
                        nc.scalar.activation(sd[:, t:t + 1], mv[:, t, 1:2],
                                             AF.Sqrt, bias=epsb[:])
                        nc.vector.reciprocal(rstd[:, t:t + 1], sd[:, t:t + 1])
                        nc.vector.tensor_scalar(
                            m2[:, t:t + 1], mv[:, t, 0:1], rstd[:, t:t + 1],
                            -1.0, OP.mult, OP.mult)
                        xh = p_xh.tile([P, D], F16, tag="xh", name=f"xh{t}")
                        with nc.allow_low_precision("fp16 activations"):
                            nc.vector.tensor_scalar(
                                xh[:], xq[:, tt, :], mv[:, t, 0:1],
                                rstd[:, t:t + 1], OP.subtract, OP.mult)
                        xhs.append(xh)
                    for j in range(DC):
                        pt = ps_t1.tile([P, 2 * P], F16, tag="t1",
                                        name=f"t1_{tg}_{j}")
                        for tt in range(2):
                            nc.tensor.transpose(
                                pt[:, tt * P:(tt + 1) * P],
                                xhs[tt][:, j * P:(j + 1) * P], ident[:])
                        dst = h1T[:, j, tg * 2 * P:(tg + 1) * 2 * P]
                        with nc.allow_low_precision("fp16 activations"):
                            if j % 2 == 0:
                                nc.scalar.activation(
                                    dst, pt[:], AF.Identity,
                                    bias=b1t[:, j:j + 1], scale=g1t[:, j:j + 1])
                            else:
                                nc.vector.tensor_scalar(
                                    dst, pt[:], g1t[:, j:j + 1],
                                    b1t[:, j:j + 1], OP.mult, OP.add)
                # QKV for this span
                for fb in range(4):
                    wsrc, half, dest, dhalf = qk_meta[fb]
                    pq = ps_qk.tile([P, SPAN], F32, tag="qk",
                                    name=f"qk{fb}_{s}")
                    for kc in range(DC):
                        nc.tensor.matmul(
                            pq[:], wsrc[:, kc, half * P:(half + 1) * P],
                            h1T[:, kc, s * SPAN:(s + 1) * SPAN],
                            start=(kc == 0), stop=(kc == DC - 1))
                    with nc.allow_low_precision("fp16 activations"):
                        nc.scalar.activation(
                            dest[:, dhalf, s * SPAN:(s + 1) * SPAN], pq[:],
                            AF.Identity, bias=bqkt[:, fb:fb + 1])
                for m in range(4 * s, 4 * s + 4):
                    pv = ps_v.tile([P, FQ], F32, tag="v", name=f"v{m}")
                    for kc in range(DC):
                        nc.tensor.matmul(
                            pv[:], h1T[:, kc, m * P:(m + 1) * P],
                            wv_t[:, kc, :],
                            start=(kc == 0), stop=(kc == DC - 1))
                    vdst = vhat[:, m, :].rearrange("p (h x) -> p h x",
                                                   x=DH + 1)
                    with nc.allow_low_precision("fp16 activations"):
                        nc.vector.tensor_tensor(
                            vdst[:, :, 0:DH],
                            pv[:].rearrange("p (h x) -> p h x", x=DH),
                            bvb[:].rearrange("p (h x) -> p h x", x=DH),
                            OP.add)
                        nc.vector.tensor_copy(vdst[:, :, DH:DH + 1], ones4[:])

        # prefetch fc1 weights into SBUF while attention computes
        for pq4 in range(4):
            m0 = pq4 * (WFC_PRE // 4)
            m1 = (pq4 + 1) * (WFC_PRE // 4)
            nc.sync.dma_start(
                wfc_sb[:, m0:m1, :, :],
                wfc_d[m0:m1].rearrange("m p c f -> p m c f"))

        # ================= Attention + proj + ReduceScatter =================
        # span-outer; scores run SKEW blocks ahead of the exp->mask->PV chain
        with tc.tile_pool(name="p_e", bufs=1) as p_e, \
             tc.tile_pool(name="p_pr", bufs=3) as p_pr, \
             tc.tile_pool(name="ps_s", bufs=4, space="PSUM") as ps_s, \
             tc.tile_pool(name="ps_pv", bufs=2, space="PSUM") as ps_pv, \
             tc.tile_pool(name="ps_pr", bufs=1, space="PSUM") as ps_pr:
            for s in range(NSPAN):
                nkb = (s + 1) * (SPAN // P)
                for h in range(FH):
                    hp, ro = h // 2, (h % 2) * DH
                    ppv = ps_pv.tile([DH + 1, SPAN], F32, tag="pv",
                                     name=f"pv{h}_{s}")
                    es = {}

                    def scores(kb, h=h, s=s, hp=hp, ro=ro, es=es):
                        pst = ps_s.tile([P, SPAN], F32, tag="sT",
                                        name=f"sT{h}_{s}_{kb}")
                        nc.tensor.matmul(
                            pst[:],
                            kT[ro:ro + DH, hp, kb * P:(kb + 1) * P],
                            qT[ro:ro + DH, hp, s * SPAN:(s + 1) * SPAN])
                        e = p_e.tile([P, SPAN], F16, tag="e", bufs=SKEW + 2,
                                     name=f"e{h}_{s}_{kb}")
                        with nc.allow_low_precision("fp16 activations"):
                            nc.scalar.activation(e[:], pst[:], AF.Exp,
                                                 scale=0.125)
                        j = kb - s * (SPAN // P)
                        if j >= 0:
                            # zero the causally-masked region in place
                            nc.gpsimd.affine_select(
                                out=e[:], in_=e[:], compare_op=OP.is_ge,
                                fill=0.0, base=-j * P, channel_multiplier=-1,
                                pattern=[[1, SPAN]])
                        es[kb] = e

                    def pv_acc(kb, h=h, nkb=nkb, ppv=ppv, es=es):
                        nc.tensor.matmul(
                            ppv[:],
                            vhat[:, kb, h * (DH + 1):(h + 1) * (DH + 1)],
                            es.pop(kb)[:], start=(kb == 0),
                            stop=(kb == nkb - 1))

                    for i in range(nkb + SKEW):
                        if i < nkb:
                            scores(i)
                        if i >= SKEW:
                            pv_acc(i - SKEW)
                    # normalize: att = pv / den  (den broadcast down 64 rows)
                    den = p_e.tile([1, SPAN], F32, tag="den", bufs=2,
                                   name=f"den{h}_{s}")
                    nc.vector.tensor_copy(den[:], ppv[DH:DH + 1, :])
                    dbs = p_e.tile([DH, SPAN], F32, tag="dbs", bufs=2,
                                   name=f"dbs{h}_{s}")
                    nc.gpsimd.partition_broadcast(dbs[:], den[:], channels=DH)
                    rbs = p_e.tile([DH, SPAN], F32, tag="rbs", bufs=2,
                                   name=f"rbs{h}_{s}")
                    nc.vector.reciprocal(rbs[:], dbs[:])
                    with nc.allow_low_precision("fp16 activations"):
                        nc.vector.tensor_tensor(
                            attT[ro:ro + DH, hp, s * SPAN:(s + 1) * SPAN],
                            ppv[0:DH, :], rbs[:], OP.mult)

                # projection partial for this token quarter, then its RS
                for mtl in range(4):
                    m = s * 4 + mtl
                    pp = ps_pr.tile([P, 2, SPAN], F32, tag="proj",
                                    name=f"pr{m}")
                    for n in range(2):
                        for kc in range(2):
                            nc.tensor.matmul(
                                pp[:, n, :], attT[:, kc, m * P:(m + 1) * P],
                                wp_t[:, kc, n * SPAN:(n + 1) * SPAN],
                                start=(kc == 0), stop=(kc == 1))
                    pe = p_pr.tile([P, D], F16, tag="pe", name=f"pe{m}")
                    with nc.allow_low_precision("fp16 rs payload"):
                        nc.vector.tensor_copy(pe[:], pp[:])
                    nc.sync.dma_start(
                        rs_in[s][mtl * P:(mtl + 1) * P, :], pe[:])
                nc.gpsimd.collective_compute(
                    "ReduceScatter", OP.add, replica_groups=GROUPS,
                    ins=[rs_in[s][:]], outs=[rs_out[s][:]])
        watt_cm.__exit__(None, None, None)

        # ================= Phase 5: residual + LN2 =================
        x2 = big.tile([P, NQ, D], F32, tag="B1", name="x2")
        h2T = big.tile([P, DC, TS], F16, tag="B2", name="h2T")
        xs_t = big.tile([P, NQ, D], F16, tag="B3", name="xs_t")
        xh2 = big.tile([P, NQ, D], F16, tag="B4", name="xh2")
        nc.sync.dma_start(xs_t[:], xs_d[:, :].rearrange("(q p) d -> p q d", p=P))

        with tc.tile_pool(name="p_l2", bufs=1) as p_l2, \
             tc.tile_pool(name="p_rs", bufs=2) as p_rs, \
             tc.tile_pool(name="ps_t2", bufs=2, space="PSUM") as ps_t2:
            bn6b = p_l2.tile([P, NQ, 2, 6], F32)
            mvb = p_l2.tile([P, NQ, 2], F32)
            sdb = p_l2.tile([P, NQ], F32)
            rstdb = p_l2.tile([P, NQ], F32)
            m2b = p_l2.tile([P, NQ], F32)
            for q in range(NQ):
                rst = p_rs.tile([P, D], F16, tag="rst", name=f"rst{q}")
                nc.sync.dma_start(rst[:], rs_out[q][:])
                nc.vector.tensor_tensor(x2[:, q, :], rst[:],
                                        xs_t[:, q, :], OP.add)
                nc.vector.tensor_tensor(x2[:, q, :], x2[:, q, :], bpb[:],
                                        OP.add)
                for a in range(2):
                    nc.vector.bn_stats(
                        bn6b[:, q, a, :], x2[:, q, a * 512:(a + 1) * 512])
                nc.vector.bn_aggr(mvb[:, q, :], bn6b[:, q, :, :])
                nc.scalar.activation(sdb[:, q:q + 1], mvb[:, q, 1:2], AF.Sqrt,
                                     bias=epsb[:])
                nc.vector.reciprocal(rstdb[:, q:q + 1], sdb[:, q:q + 1])
                nc.vector.tensor_scalar(
                    m2b[:, q:q + 1], mvb[:, q, 0:1], rstdb[:, q:q + 1], -1.0,
                    OP.mult, OP.mult)
                with nc.allow_low_precision("fp16 activations"):
                    nc.scalar.activation(
                        xh2[:, q, :], x2[:, q, :], AF.Identity,
                        bias=m2b[:, q:q + 1], scale=rstdb[:, q:q + 1])
            for q in range(NQ):
                for j in range(DC):
                    pt = ps_t2.tile([P, P], F16, tag="t2",
                                    name=f"t2_{q}_{j}")
                    nc.tensor.transpose(
                        pt[:], xh2[:, q, j * P:(j + 1) * P], ident[:])
                    with nc.allow_low_precision("fp16 activations"):
                        nc.scalar.activation(
                            h2T[:, j, q * P:(q + 1) * P], pt[:],
                            AF.Identity, bias=b2t[:, j:j + 1],
                            scale=g2t[:, j:j + 1])

        # ================= Phase 6: FFN =================
        aT = big.tile([P, HID // P, TS], F16, tag="A", name="aT")
        with tc.tile_pool(name="p_wf", bufs=4) as p_wf, \
             tc.tile_pool(name="ps_f1", bufs=2, space="PSUM") as ps_f1:
            for m in range(HID // P):
                if m < WFC_PRE:
                    stat = lambda kc, m=m: wfc_sb[:, m, kc, :]
                else:
                    wmt = p_wf.tile([P, DC, P], F16, tag="wfc",
                                    name=f"wfc{m}")
                    nc.sync.dma_start(wmt[:], wfc_d[m])
                    stat = lambda kc, w=wmt: w[:, kc, :]
                pf = ps_f1.tile([P, TS], F32, tag="f1", name=f"f1_{m}")
                for kc in range(DC):
                    nc.tensor.matmul(pf[:], stat(kc), h2T[:, kc, :],
                                     start=(kc == 0), stop=(kc == DC - 1))
                with nc.allow_low_precision("fp16 activations"):
                    nc.scalar.activation(aT[:, m, :], pf[:], AF.Gelu,
                                         bias=bfct[:, m:m + 1])

        with tc.tile_pool(name="p_w2", bufs=6) as p_w2, \
             tc.tile_pool(name="p_ot", bufs=3) as p_ot, \
             tc.tile_pool(name="ps_f2", bufs=4, space="PSUM") as ps_f2:
            p4s = [ps_f2.tile([P, 2, SPAN], F32, tag="f2", name=f"f2_{mt}")
                   for mt in range(NQ)]
            for kc in range(HID // P):
                w2 = p_w2.tile([P, D], F16, tag="w2", name=f"w2_{kc}")
                nc.sync.dma_start(w2[:], wfc2_d[kc * P:(kc + 1) * P, :])
                for mt in range(NQ):
                    for n in range(2):
                        nc.tensor.matmul(
                            p4s[mt][:, n, :],
                            aT[:, kc, mt * P:(mt + 1) * P],
                            w2[:, n * SPAN:(n + 1) * SPAN],
                            start=(kc == 0), stop=(kc == HID // P - 1))
            for mt in range(NQ):
                for n in range(2):
                    ot = p_ot.tile([P, SPAN], F32, tag="ot",
                                   name=f"ot{n}_{mt}")
                    nc.vector.tensor_tensor(
                        ot[:], p4s[mt][:, n, :],
                        x2[:, mt, n * SPAN:(n + 1) * SPAN], OP.add)
                    nc.vector.tensor_tensor(
                        ot[:], ot[:],
                        bfc2b[:, n * SPAN:(n + 1) * SPAN], OP.add)
                    nc.sync.dma_start(
                        out_d[mt * P:(mt + 1) * P, n * SPAN:(n + 1) * SPAN],
                        ot[:])

        big_cm.__exit__(None, None, None)
        cst_cm.__exit__(None, None, None)

    nc.finalize()
    return nc


def shard_inputs(inputs):
    """Full inputs -> per-core in_maps (8 cores)."""
    f = lambda a: np.ascontiguousarray(np.asarray(a, dtype=np.float32))
    h = lambda a: np.ascontiguousarray(np.asarray(a, dtype=np.float16))
    x = f(inputs["x"])
    w_attn, b_attn = f(inputs["w_attn"]), f(inputs["b_attn"])
    w_proj, b_proj = f(inputs["w_proj"]), f(inputs["b_proj"])
    ln1_g, ln1_b = f(inputs["ln1_g"]), f(inputs["ln1_b"])
    ln2_g, ln2_b = f(inputs["ln2_g"]), f(inputs["ln2_b"])
    w_fc, b_fc = f(inputs["w_fc"]), f(inputs["b_fc"])
    w_fc2, b_fc2 = f(inputs["w_fc2"]), f(inputs["b_fc2"])

    g1t = np.ascontiguousarray(ln1_g.reshape(DC, P).T)
    b1t = np.ascontiguousarray(ln1_b.reshape(DC, P).T)
    g2t = np.ascontiguousarray(ln2_g.reshape(DC, P).T)
    b2t = np.ascontiguousarray(ln2_b.reshape(DC, P).T)
    wfc_r = h(w_fc.reshape(DC, P, HID // P, P).transpose(2, 1, 0, 3))
    bfct = np.ascontiguousarray(b_fc.reshape(HID // P, P).T)
    bpb = np.ascontiguousarray(np.broadcast_to(b_proj.reshape(1, D), (P, D)))
    bfc2b = np.ascontiguousarray(np.broadcast_to(b_fc2.reshape(1, D), (P, D)))
    wfc2_h = h(w_fc2)

    in_maps = []
    for c in range(8):
        g, tp = c // 4, c % 4
        sl = slice(tp * FQ, (tp + 1) * FQ)
        bq = b_attn[0 * D:1 * D][sl]
        bk = b_attn[1 * D:2 * D][sl]
        bv = b_attn[2 * D:3 * D][sl]
        bqk = np.concatenate([bq.reshape(2, P).T, bk.reshape(2, P).T], axis=1)
        bvb = np.broadcast_to(bv.reshape(1, FQ), (P, FQ))
        xs = np.concatenate(
            [x[g, q * SPAN + tp * P: q * SPAN + (tp + 1) * P]
             for q in range(NQ)], axis=0)
        in_maps.append({
            "x": h(x[g]),
            "xs": h(xs),
            "wq": h(w_attn[:, 0 * D:1 * D][:, sl]),
            "wk": h(w_attn[:, 1 * D:2 * D][:, sl]),
            "wv": h(w_attn[:, 2 * D:3 * D][:, sl]),
            "bqk": np.ascontiguousarray(bqk),
            "bvb": np.ascontiguousarray(bvb),
            "wp": h(w_proj[sl, :]),
            "bpb": bpb,
            "g1": g1t, "b1": b1t, "g2": g2t, "b2": b2t,
            "wfc": wfc_r, "bfc": bfct,
            "wfc2": wfc2_h, "bfc2b": bfc2b,
        })
    return in_maps


def assemble(results):
    out = np.empty((2, T, D), dtype=np.float32)
    for c in range(8):
        g, tp = c // 4, c % 4
        r = np.asarray(results[c]["out"])
        for q in range(NQ):
            out[g, q * SPAN + tp * P: q * SPAN + (tp + 1) * P] = \
                r[q * P:(q + 1) * P]
    return out


_NC = None


def kernel(**inputs):
    global _NC
    if _NC is None:
        _NC = build_nc()
    in_maps = shard_inputs(inputs)
    res = run_bass_kernel_spmd(_NC, in_maps, list(range(8)))
    return assemble(res.results)


# revision 31
# speedup vs baseline: 1.1070x; 1.0384x over previous
"""Trainium2 Bass kernel for a GPT-style decoder block (B=2, T=2048, d=1024,
16 heads, FFN 4096), distributed over 8 NeuronCores.

Sharding: DP2 (batch) x TP4 (4 heads + proj-row split per core). The single
collective is a per-token-quarter ReduceScatter of the attention projection
partials over each 4-core group; after it, every core owns its token strips
and runs LN2+FFN (full hidden dim) on just those, writing its 512-token
output slice. The last quarter's ReduceScatter is split in two so only a
256-token collective remains on the critical tail.

v3: fp16 matmul operands throughout (PSUM stays fp32); LN1+QKV pipelined
per 512-token span; attention scores run 3 blocks ahead of the
exp->mask->PV chain so the PE never waits on the softmax; softmax
normalization via partition_broadcast + DVE divide (no PE/ACT involvement);
fc1 weights preloaded to SBUF during attention.

Self-contained: hardcodes all shapes; no sibling imports.
"""
import numpy as np

import concourse.bacc as bacc
import concourse.mybir as mybir
import concourse.tile as tile
from concourse.bass_utils import run_bass_kernel_spmd
from concourse.masks import make_identity

F32 = mybir.dt.float32
F16 = mybir.dt.float16
AF = mybir.ActivationFunctionType
OP = mybir.AluOpType

P = 128
T = 2048          # tokens per batch element
D = 1024          # embed dim
NT = T // P       # 16 token tiles
DC = D // P       # 8 d-chunks
FH = 4            # heads per core
DH = 64           # head dim
FQ = 256          # q (=k=v) features per core
HID = 4096        # full FFN hidden
TS = 512          # token slice per core
NQ = 4            # token quarters
SPAN = 512        # attention query span
NSPAN = T // SPAN
EPS = 1e-5
GROUPS = [[0, 1, 2, 3], [4, 5, 6, 7]]
WFC_PRE = 24      # fc1 hid-tiles preloaded to SBUF (rest streamed)
SKEW = 3          # scores run this many blocks ahead of PV


def build_nc():
    nc = bacc.Bacc(None, target_bir_lowering=False)

    # ---- external I/O ----
    x_d = nc.dram_tensor("x", [T, D], F16, kind="ExternalInput")
    xs_d = nc.dram_tensor("xs", [TS, D], F16, kind="ExternalInput")
    wq_d = nc.dram_tensor("wq", [D, FQ], F16, kind="ExternalInput")
    wk_d = nc.dram_tensor("wk", [D, FQ], F16, kind="ExternalInput")
    wv_d = nc.dram_tensor("wv", [D, FQ], F16, kind="ExternalInput")
    bqk_d = nc.dram_tensor("bqk", [P, 4], F32, kind="ExternalInput")
    bvb_d = nc.dram_tensor("bvb", [P, FQ], F32, kind="ExternalInput")
    wp_d = nc.dram_tensor("wp", [FQ, D], F16, kind="ExternalInput")
    g1_d = nc.dram_tensor("g1", [P, DC], F32, kind="ExternalInput")
    b1_d = nc.dram_tensor("b1", [P, DC], F32, kind="ExternalInput")
    g2_d = nc.dram_tensor("g2", [P, DC], F32, kind="ExternalInput")
    b2_d = nc.dram_tensor("b2", [P, DC], F32, kind="ExternalInput")
    wfc_d = nc.dram_tensor("wfc", [HID // P, P, DC, P], F16,
                           kind="ExternalInput")
    bfc_d = nc.dram_tensor("bfc", [P, HID // P], F32, kind="ExternalInput")
    wfc2_d = nc.dram_tensor("wfc2", [HID, D], F16, kind="ExternalInput")
    bfc2b_d = nc.dram_tensor("bfc2b", [P, D], F32, kind="ExternalInput")
    out_d = nc.dram_tensor("out", [TS, D], F32, kind="ExternalOutput")

    rs_in = [nc.dram_tensor(f"rs_in{q}", [SPAN, D], F16) for q in range(NQ)]
    rs_out = [nc.dram_tensor(f"rs_out{q}", [P, D], F16) for q in range(NQ)]

    with tile.TileContext(nc) as tc:
        pxq_cm = tc.tile_pool(name="p_xq", bufs=2)
        p_xq = pxq_cm.__enter__()
        cst_cm = tc.tile_pool(name="cst", bufs=1)
        cst = cst_cm.__enter__()
        big_cm = tc.tile_pool(name="big", bufs=1)
        big = big_cm.__enter__()

        # ---- first x tiles before anything else hits the DMA queues ----
        xsrc0 = x_d[:, :].rearrange("(t p) d -> p t d", p=P)
        xq01 = []
        for tg in range(2):
            xq = p_xq.tile([P, 2, D], F16, tag="xq", name=f"xq{tg}")
            for tt in range(2):
                nc.sync.dma_start(xq[:, tt:tt + 1, :],
                                  xsrc0[:, tg * 2 + tt:tg * 2 + tt + 1, :])
            xq01.append(xq)

        # ---- constants / small params ----
        identf = cst.tile([P, P], F32)
        make_identity(nc, identf[:])
        ident = cst.tile([P, P], F16)
        nc.vector.tensor_copy(ident[:], identf[:])
        ones4 = cst.tile([P, FH, 1], F16)
        with nc.allow_low_precision("exact value 1.0"):
            nc.gpsimd.memset(ones4[:], 1.0)
        epsb = cst.tile([P, 1], F32)
        nc.gpsimd.memset(epsb[:], EPS)

        bqkt = cst.tile([P, 4], F32)
        nc.sync.dma_start(bqkt[:], bqk_d[:])
        g1t = cst.tile([P, DC], F32)
        nc.sync.dma_start(g1t[:], g1_d[:])
        b1t = cst.tile([P, DC], F32)
        nc.sync.dma_start(b1t[:], b1_d[:])
        g2t = cst.tile([P, DC], F32)
        nc.sync.dma_start(g2t[:], g2_d[:])
        b2t = cst.tile([P, DC], F32)
        nc.sync.dma_start(b2t[:], b2_d[:])
        bfct = cst.tile([P, HID // P], F32)
        nc.sync.dma_start(bfct[:], bfc_d[:])
        bvb = cst.tile([P, FQ], F32)
        nc.sync.dma_start(bvb[:], bvb_d[:])
        bfc2b = cst.tile([P, D], F32)
        nc.sync.dma_start(bfc2b[:], bfc2b_d[:])

        # ---- resident weights ----
        watt_cm = tc.tile_pool(name="watt", bufs=1)
        watt = watt_cm.__enter__()
        wq_t = watt.tile([P, DC, FQ], F16)
        wk_t = watt.tile([P, DC, FQ], F16)
        wv_t = watt.tile([P, DC, FQ], F16)
        wp_t = watt.tile([P, 2, D], F16)
        nc.sync.dma_start(wq_t[:], wq_d[:, :].rearrange("(c p) f -> p c f", p=P))
        nc.sync.dma_start(wk_t[:], wk_d[:, :].rearrange("(c p) f -> p c f", p=P))
        nc.sync.dma_start(wv_t[:], wv_d[:, :].rearrange("(c p) f -> p c f", p=P))
        nc.sync.dma_start(wp_t[:], wp_d[:, :].rearrange("(c p) f -> p c f", p=P))
        # fc1 weight, mostly preloaded (DMA issued at attention start);
        # lives in `big` so it survives until the FFN phase
        wfc_sb = big.tile([P, WFC_PRE, DC, P], F16, name="wfc_sb")

        # ---- big tag-shared tiles (phase 1-4 lives) ----
        h1T = big.tile([P, DC, T], F16, tag="A", name="h1T")
        qT = big.tile([P, 2, T], F16, tag="B1", name="qT")
        kT = big.tile([P, 2, T], F16, tag="B2", name="kT")
        vhat = big.tile([P, NT, FH * (DH + 1)], F16, tag="B3", name="vhat")
        attT = big.tile([P, 2, T], F16, tag="B4", name="attT")

        # ============ Front: LN1 + transpose + QKV, pipelined per span ========
        qk_meta = [(wq_t, 0, qT, 0), (wq_t, 1, qT, 1),
                   (wk_t, 0, kT, 0), (wk_t, 1, kT, 1)]
        with tc.tile_pool(name="p_ln", bufs=1) as p_ln, \
             tc.tile_pool(name="p_xh", bufs=3) as p_xh, \
             tc.tile_pool(name="ps_t1", bufs=2, space="PSUM") as ps_t1, \
             tc.tile_pool(name="ps_qk", bufs=3, space="PSUM") as ps_qk, \
             tc.tile_pool(name="ps_v", bufs=2, space="PSUM") as ps_v:
            bn6 = p_ln.tile([P, NT, 2, 6], F32)
            mv = p_ln.tile([P, NT, 2], F32)
            sd = p_ln.tile([P, NT], F32)
            rstd = p_ln.tile([P, NT], F32)
            m2 = p_ln.tile([P, NT], F32)
            xsrc = x_d[:, :].rearrange("(t p) d -> p t d", p=P)
            for s in range(NSPAN):
                for tgl in range(2):
                    tg = 2 * s + tgl
                    if tg < 2:
                        xq = xq01[tg]
                    else:
                        xq = p_xq.tile([P, 2, D], F16, tag="xq",
                                       name=f"xq{tg}")
                        nc.sync.dma_start(xq[:],
                                          xsrc[:, tg * 2:(tg + 1) * 2, :])
                    xhs = []
                    for tt in range(2):
                        t = tg * 2 + tt
                        for a in range(2):
                            nc.vector.bn_stats(
                                bn6[:, t, a, :],
                                xq[:, tt, a * 512:(a + 1) * 512])
                        nc.vector.bn_aggr(mv[:, t, :], bn6[:, t, :, :])
                        nc.scalar.activation(sd[:, t:t + 1], mv[:, t, 1:2],
                                             AF.Sqrt, bias=epsb[:])
                        nc.vector.reciprocal(rstd[:, t:t + 1], sd[:, t:t + 1])
                        nc.vector.tensor_scalar(
                            m2[:, t:t + 1], mv[:, t, 0:1], rstd[:, t:t + 1],
                            -1.0, OP.mult, OP.mult)
                        xh = p_xh.tile([P, D], F16, tag="xh", name=f"xh{t}")
                        with nc.allow_low_precision("fp16 activations"):
                            nc.vector.tensor_scalar(
                                xh[:], xq[:, tt, :], mv[:, t, 0:1],
                                rstd[:, t:t + 1], OP.subtract, OP.mult)
                        xhs.append(xh)
                    for j in range(DC):
                        pt = ps_t1.tile([P, 2 * P], F16, tag="t1",
                                        name=f"t1_{tg}_{j}")
                        for tt in range(2):
                            nc.tensor.transpose(
                                pt[:, tt * P:(tt + 1) * P],
                                xhs[tt][:, j * P:(j + 1) * P], ident[:])
                        dst = h1T[:, j, tg * 2 * P:(tg + 1) * 2 * P]
                        with nc.allow_low_precision("fp16 activations"):
                            if j % 2 == 0:
                                nc.scalar.activation(
                                    dst, pt[:], AF.Identity,
                                    bias=b1t[:, j:j + 1], scale=g1t[:, j:j + 1])
                            else:
                                nc.vector.tensor_scalar(
                                    dst, pt[:], g1t[:, j:j + 1],
                                    b1t[:, j:j + 1], OP.mult, OP.add)
                # QKV for this span
                for fb in range(4):
                    wsrc, half, dest, dhalf = qk_meta[fb]
                    pq = ps_qk.tile([P, SPAN], F32, tag="qk",
                                    name=f"qk{fb}_{s}")
                    for kc in range(DC):
                        nc.tensor.matmul(
                            pq[:], wsrc[:, kc, half * P:(half + 1) * P],
                            h1T[:, kc, s * SPAN:(s + 1) * SPAN],
                            start=(kc == 0), stop=(kc == DC - 1))
                    with nc.allow_low_precision("fp16 activations"):
                        nc.scalar.activation(
                            dest[:, dhalf, s * SPAN:(s + 1) * SPAN], pq[:],
                            AF.Identity, bias=bqkt[:, fb:fb + 1])
                for m in range(4 * s, 4 * s + 4):
                    pv = ps_v.tile([P, FQ], F32, tag="v", name=f"v{m}")
                    for kc in range(DC):
                        nc.tensor.matmul(
                            pv[:], h1T[:, kc, m * P:(m + 1) * P],
                            wv_t[:, kc, :],
                            start=(kc == 0), stop=(kc == DC - 1))
                    vdst = vhat[:, m, :].rearrange("p (h x) -> p h x",
                                                   x=DH + 1)
                    with nc.allow_low_precision("fp16 activations"):
                        nc.vector.tensor_tensor(
                            vdst[:, :, 0:DH],
                            pv[:].rearrange("p (h x) -> p h x", x=DH),
                            bvb[:].rearrange("p (h x) -> p h x", x=DH),
                            OP.add)
                        nc.vector.tensor_copy(vdst[:, :, DH:DH + 1], ones4[:])

        # prefetch fc1 weights into SBUF while attention computes
        for pq4 in range(4):
            m0 = pq4 * (WFC_PRE // 4)
            m1 = (pq4 + 1) * (WFC_PRE // 4)
            nc.sync.dma_start(
                wfc_sb[:, m0:m1, :, :],
                wfc_d[m0:m1].rearrange("m p c f -> p m c f"))

        # ================= Attention + proj + ReduceScatter =================
        # span-outer; scores run SKEW blocks ahead of the exp->mask->PV chain
        with tc.tile_pool(name="p_e", bufs=1) as p_e, \
             tc.tile_pool(name="p_pr", bufs=3) as p_pr, \
             tc.tile_pool(name="ps_s", bufs=4, space="PSUM") as ps_s, \
             tc.tile_pool(name="ps_pv", bufs=2, space="PSUM") as ps_pv, \
             tc.tile_pool(name="ps_pr", bufs=1, space="PSUM") as ps_pr:
            for s in range(NSPAN):
                nkb = (s + 1) * (SPAN // P)
                for h in range(FH):
                    hp, ro = h // 2, (h % 2) * DH
                    ppv = ps_pv.tile([DH + 1, SPAN], F32, tag="pv",
                                     name=f"pv{h}_{s}")
                    es = {}

                    def scores(kb, h=h, s=s, hp=hp, ro=ro, es=es):
                        pst = ps_s.tile([P, SPAN], F32, tag="sT",
                                        name=f"sT{h}_{s}_{kb}")
                        nc.tensor.matmul(
                            pst[:],
                            kT[ro:ro + DH, hp, kb * P:(kb + 1) * P],
                            qT[ro:ro + DH, hp, s * SPAN:(s + 1) * SPAN])
                        e = p_e.tile([P, SPAN], F16, tag="e", bufs=SKEW + 2,
                                     name=f"e{h}_{s}_{kb}")
                        with nc.allow_low_precision("fp16 activations"):
                            nc.scalar.activation(e[:], pst[:], AF.Exp,
                                                 scale=0.125)
                        j = kb - s * (SPAN // P)
                        if j >= 0:
                            # zero the causally-masked region in place
                            nc.gpsimd.affine_select(
                                out=e[:], in_=e[:], compare_op=OP.is_ge,
                                fill=0.0, base=-j * P, channel_multiplier=-1,
                                pattern=[[1, SPAN]])
                        es[kb] = e

                    def pv_acc(kb, h=h, nkb=nkb, ppv=ppv, es=es):
                        nc.tensor.matmul(
                            ppv[:],
                            vhat[:, kb, h * (DH + 1):(h + 1) * (DH + 1)],
                            es.pop(kb)[:], start=(kb == 0),
                            stop=(kb == nkb - 1))

                    for i in range(nkb + SKEW):
                        if i < nkb:
                            scores(i)
                        if i >= SKEW:
                            pv_acc(i - SKEW)
                    # normalize: att = pv / den  (den broadcast down 64 rows)
                    den = p_e.tile([1, SPAN], F32, tag="den", bufs=2,
                                   name=f"den{h}_{s}")
                    nc.vector.tensor_copy(den[:], ppv[DH:DH + 1, :])
                    dbs = p_e.tile([DH, SPAN], F32, tag="dbs", bufs=2,
                                   name=f"dbs{h}_{s}")
                    nc.gpsimd.partition_broadcast(dbs[:], den[:], channels=DH)
                    rbs = p_e.tile([DH, SPAN], F32, tag="rbs", bufs=2,
                                   name=f"rbs{h}_{s}")
                    nc.vector.reciprocal(rbs[:], dbs[:])
                    with nc.allow_low_precision("fp16 activations"):
                        nc.vector.tensor_tensor(
                            attT[ro:ro + DH, hp, s * SPAN:(s + 1) * SPAN],
                            ppv[0:DH, :], rbs[:], OP.mult)

                # projection partial for this token quarter, then its RS
                for mtl in range(4):
                    m = s * 4 + mtl
                    pp = ps_pr.tile([P, 2, SPAN], F32, tag="proj",
                                    name=f"pr{m}")
                    for n in range(2):
                        for kc in range(2):
                            nc.tensor.matmul(
                                pp[:, n, :], attT[:, kc, m * P:(m + 1) * P],
                                wp_t[:, kc, n * SPAN:(n + 1) * SPAN],
                                start=(kc == 0), stop=(kc == 1))
                    pe = p_pr.tile([P, D], F16, tag="pe", name=f"pe{m}")
                    with nc.allow_low_precision("fp16 rs payload"):
                        nc.vector.tensor_copy(pe[:], pp[:])
                    nc.sync.dma_start(
                        rs_in[s][mtl * P:(mtl + 1) * P, :], pe[:])
                nc.gpsimd.collective_compute(
                    "ReduceScatter", OP.add, replica_groups=GROUPS,
                    ins=[rs_in[s][:]], outs=[rs_out[s][:]])
        watt_cm.__exit__(None, None, None)

        # ================= Phase 5: residual + LN2 =================
        x2 = big.tile([P, NQ, D], F32, tag="B1", name="x2")
        h2T = big.tile([P, DC, TS], F16, tag="B2", name="h2T")
        xs_t = big.tile([P, NQ, D], F16, tag="B3", name="xs_t")
        xh2 = big.tile([P, NQ, D], F16, tag="B4", name="xh2")
        nc.sync.dma_start(xs_t[:], xs_d[:, :].rearrange("(q p) d -> p q d", p=P))

        with tc.tile_pool(name="p_l2", bufs=1) as p_l2, \
             tc.tile_pool(name="p_rs", bufs=2) as p_rs, \
             tc.tile_pool(name="ps_t2", bufs=2, space="PSUM") as ps_t2:
            bn6b = p_l2.tile([P, NQ, 2, 6], F32)
            mvb = p_l2.tile([P, NQ, 2], F32)
            sdb = p_l2.tile([P, NQ], F32)
            rstdb = p_l2.tile([P, NQ], F32)
            m2b = p_l2.tile([P, NQ], F32)
            for q in range(NQ):
                rst = p_rs.tile([P, D], F16, tag="rst", name=f"rst{q}")
                nc.sync.dma_start(rst[:], rs_out[q][:])
                nc.vector.tensor_tensor(x2[:, q, :], rst[:],
                                        xs_t[:, q, :], OP.add)
                for a in range(2):
                    nc.vector.bn_stats(
                        bn6b[:, q, a, :], x2[:, q, a * 512:(a + 1) * 512])
                nc.vector.bn_aggr(mvb[:, q, :], bn6b[:, q, :, :])
                nc.scalar.activation(sdb[:, q:q + 1], mvb[:, q, 1:2], AF.Sqrt,
                                     bias=epsb[:])
                nc.vector.reciprocal(rstdb[:, q:q + 1], sdb[:, q:q + 1])
                nc.vector.tensor_scalar(
                    m2b[:, q:q + 1], mvb[:, q, 0:1], rstdb[:, q:q + 1], -1.0,
                    OP.mult, OP.mult)
                with nc.allow_low_precision("fp16 activations"):
                    nc.scalar.activation(
                        xh2[:, q, :], x2[:, q, :], AF.Identity,
                        bias=m2b[:, q:q + 1], scale=rstdb[:, q:q + 1])
            for q in range(NQ):
                for j in range(DC):
                    pt = ps_t2.tile([P, P], F16, tag="t2",
                                    name=f"t2_{q}_{j}")
                    nc.tensor.transpose(
                        pt[:], xh2[:, q, j * P:(j + 1) * P], ident[:])
                    with nc.allow_low_precision("fp16 activations"):
                        nc.scalar.activation(
                            h2T[:, j, q * P:(q + 1) * P], pt[:],
                            AF.Identity, bias=b2t[:, j:j + 1],
                            scale=g2t[:, j:j + 1])

        # ================= Phase 6: FFN =================
        aT = big.tile([P, HID // P, TS], F16, tag="A", name="aT")
        with tc.tile_pool(name="p_wf", bufs=4) as p_wf, \
             tc.tile_pool(name="ps_f1", bufs=2, space="PSUM") as ps_f1:
            for m in range(HID // P):
                if m < WFC_PRE:
                    stat = lambda kc, m=m: wfc_sb[:, m, kc, :]
                else:
                    wmt = p_wf.tile([P, DC, P], F16, tag="wfc",
                                    name=f"wfc{m}")
                    nc.sync.dma_start(wmt[:], wfc_d[m])
                    stat = lambda kc, w=wmt: w[:, kc, :]
                pf = ps_f1.tile([P, TS], F32, tag="f1", name=f"f1_{m}")
                for kc in range(DC):
                    nc.tensor.matmul(pf[:], stat(kc), h2T[:, kc, :],
                                     start=(kc == 0), stop=(kc == DC - 1))
                with nc.allow_low_precision("fp16 activations"):
                    nc.scalar.activation(aT[:, m, :], pf[:], AF.Gelu,
                                         bias=bfct[:, m:m + 1])

        with tc.tile_pool(name="p_w2", bufs=6) as p_w2, \
             tc.tile_pool(name="p_ot", bufs=3) as p_ot, \
             tc.tile_pool(name="ps_f2", bufs=4, space="PSUM") as ps_f2:
            p4s = [ps_f2.tile([P, 2, SPAN], F32, tag="f2", name=f"f2_{mt}")
                   for mt in range(NQ)]
            for kc in range(HID // P):
                w2 = p_w2.tile([P, D], F16, tag="w2", name=f"w2_{kc}")
                nc.sync.dma_start(w2[:], wfc2_d[kc * P:(kc + 1) * P, :])
                for mt in range(NQ):
                    for n in range(2):
                        nc.tensor.matmul(
                            p4s[mt][:, n, :],
                            aT[:, kc, mt * P:(mt + 1) * P],
                            w2[:, n * SPAN:(n + 1) * SPAN],
                            start=(kc == 0), stop=(kc == HID // P - 1))
            for mt in range(NQ):
                for n in range(2):
                    ot = p_ot.tile([P, SPAN], F32, tag="ot",
                                   name=f"ot{n}_{mt}")
                    nc.vector.tensor_tensor(
                        ot[:], p4s[mt][:, n, :],
                        x2[:, mt, n * SPAN:(n + 1) * SPAN], OP.add)
                    nc.vector.tensor_tensor(
                        ot[:], ot[:],
                        bfc2b[:, n * SPAN:(n + 1) * SPAN], OP.add)
                    nc.sync.dma_start(
                        out_d[mt * P:(mt + 1) * P, n * SPAN:(n + 1) * SPAN],
                        ot[:])

        big_cm.__exit__(None, None, None)
        cst_cm.__exit__(None, None, None)
        pxq_cm.__exit__(None, None, None)

    nc.finalize()
    return nc


def shard_inputs(inputs):
    """Full inputs -> per-core in_maps (8 cores)."""
    f = lambda a: np.ascontiguousarray(np.asarray(a, dtype=np.float32))
    h = lambda a: np.ascontiguousarray(np.asarray(a, dtype=np.float16))
    x = f(inputs["x"])
    w_attn, b_attn = f(inputs["w_attn"]), f(inputs["b_attn"])
    w_proj, b_proj = f(inputs["w_proj"]), f(inputs["b_proj"])
    ln1_g, ln1_b = f(inputs["ln1_g"]), f(inputs["ln1_b"])
    ln2_g, ln2_b = f(inputs["ln2_g"]), f(inputs["ln2_b"])
    w_fc, b_fc = f(inputs["w_fc"]), f(inputs["b_fc"])
    w_fc2, b_fc2 = f(inputs["w_fc2"]), f(inputs["b_fc2"])

    g1t = np.ascontiguousarray(ln1_g.reshape(DC, P).T)
    b1t = np.ascontiguousarray(ln1_b.reshape(DC, P).T)
    g2t = np.ascontiguousarray(ln2_g.reshape(DC, P).T)
    b2t = np.ascontiguousarray(ln2_b.reshape(DC, P).T)
    wfc_r = h(w_fc.reshape(DC, P, HID // P, P).transpose(2, 1, 0, 3))
    bfct = np.ascontiguousarray(b_fc.reshape(HID // P, P).T)
    bfc2b = np.ascontiguousarray(np.broadcast_to(b_fc2.reshape(1, D), (P, D)))
    wfc2_h = h(w_fc2)

    in_maps = []
    for c in range(8):
        g, tp = c // 4, c % 4
        sl = slice(tp * FQ, (tp + 1) * FQ)
        bq = b_attn[0 * D:1 * D][sl]
        bk = b_attn[1 * D:2 * D][sl]
        bv = b_attn[2 * D:3 * D][sl]
        bqk = np.concatenate([bq.reshape(2, P).T, bk.reshape(2, P).T], axis=1)
        bvb = np.broadcast_to(bv.reshape(1, FQ), (P, FQ))
        xs = np.concatenate(
            [x[g, q * SPAN + tp * P: q * SPAN + (tp + 1) * P]
             for q in range(NQ)], axis=0) + b_proj.reshape(1, D)
        in_maps.append({
            "x": h(x[g]),
            "xs": h(xs),
            "wq": h(w_attn[:, 0 * D:1 * D][:, sl]),
            "wk": h(w_attn[:, 1 * D:2 * D][:, sl]),
            "wv": h(w_attn[:, 2 * D:3 * D][:, sl]),
            "bqk": np.ascontiguousarray(bqk),
            "bvb": np.ascontiguousarray(bvb),
            "wp": h(w_proj[sl, :]),
            "g1": g1t, "b1": b1t, "g2": g2t, "b2": b2t,
            "wfc": wfc_r, "bfc": bfct,
            "wfc2": wfc2_h, "bfc2b": bfc2b,
        })
    return in_maps


def assemble(results):
    out = np.empty((2, T, D), dtype=np.float32)
    for c in range(8):
        g, tp = c // 4, c % 4
        r = np.asarray(results[c]["out"])
        for q in range(NQ):
            out[g, q * SPAN + tp * P: q * SPAN + (tp + 1) * P] = \
                r[q * P:(q + 1) * P]
    return out


_NC = None


def kernel(**inputs):
    global _NC
    if _NC is None:
        _NC = build_nc()
    in_maps = shard_inputs(inputs)
    res = run_bass_kernel_spmd(_NC, in_maps, list(range(8)))
    return assemble(res.results)
